# revision 1
# baseline (speedup 1.0000x reference)
"""Trainium2 Bass kernel for the CIDER GNN-message-passing head.

Math (algebraically reduced from the reference; validated to ~7e-7 rel):
  neigh[b]  = sum_{h<96} hist[b,h,:]            (mean folded into W_l scale)
  c[b]      = neigh[b] @ (W_l.T/96) + b_l
  urep[b]   = hist[b,0,:] @ W_r.T + c[b]
  Q[b]      = urep[b] @ W_Q.T + b_Q
  v[b]      = Q[b] @ W_K
  u[b]      = v[b] @ W_r
  sc[b,h]   = hist[b,h,:] . u[b]                (c.v shift dropped: softmax-invariant)
  e         = exp((sc - max_h sc)/16),  Z = sum_h e
  hbar[b]   = sum_h e[b,h] * hist[b,h,:]
  out[b]    = (hbar[b] @ W_r.T)/Z[b] + c[b]
  output    = broadcast out over the candidate dim (reference output is
              provably constant along it).

Sharding: pure data parallel, batch 96 -> 12 samples on each of 8 cores;
weights replicated; no collectives.

Layout/perf choices:
  - hist ships host-transposed (d-major) so it lands d-on-partitions with
    zero on-device transposes; data and weights ship as float16 (10-bit
    mantissa, comparable to the PE's fp32r path) halving HBM traffic and
    running the PE at full rate with fp32 PSUM accumulation.
  - W_Q.T @ W_K @ W_r is composed on the host: the Q and v stages (and all
    their transposes) disappear, as does any need for W_r in natural
    orientation on device.
  - scores come from a batched cross-product matmul (all samples at once);
    the diagonal is extracted with a masked reduce (engines cannot address
    single unaligned partitions).
  - softmax scale 1/16 folds into the Exp activation; 1/Z folds into the
    final fused (psum*recip + c) epilogue; softmax's per-sample shift
    c.v cancels and is dropped.
  - the exp-weighted history sum runs as DVE mul + per-k-pair reduces with
    the final W_r.T matmuls interleaved; junk "warmer" matmuls keep the PE
    HAM clock-gate open across the softmax stretch.
"""

import os
import sys

for _p in ("/opt/trn_rl_repo", "/root/.axon_site/_ro/trn_rl_repo"):
    if os.path.isdir(_p) and _p not in sys.path:
        sys.path.insert(0, _p)

import numpy as np

import concourse.bacc as bacc
import concourse.tile as tile
from concourse import mybir
from concourse.bass_utils import run_bass_kernel_spmd

B, H, NCAND, D, A = 96, 100, 128, 768, 256
NCORES = 8
BC = B // NCORES          # 12 samples per core
DC = D // 128             # 6 chunks of the 768-dim
AC = A // 128             # 2 chunks of the 256-dim
NH = D // 2               # 384-wide matmul halves (one PSUM bank each)
NG = 3                    # sample groups of 4 (separate tiles => finer deps)
GS = BC // NG

F32 = mybir.dt.float32
F32R = mybir.dt.float32r
F16 = mybir.dt.float16
DT = F16            # PE-operand dtype (shipped data); fp32 stays on-chip
X = mybir.AxisListType.X
X2 = mybir.AxisListType.XY
ALU = mybir.AluOpType
ACTF = mybir.ActivationFunctionType

_CACHE = {}


def _build():
    nc = bacc.Bacc(
        "TRN2",
        target_bir_lowering=False,
        debug=False,
        enable_asserts=True,
        num_devices=NCORES,
    )
    hist_d = nc.dram_tensor("hist", [D, BC * H], DT, kind="ExternalInput")
    wlt_d = nc.dram_tensor("wlt", [D, D], DT, kind="ExternalInput")   # W_l.T/96
    wrt_d = nc.dram_tensor("wrt", [D, D], DT, kind="ExternalInput")   # W_r.T
    wqkr_d = nc.dram_tensor("wqkr", [D, D], DT, kind="ExternalInput")  # W_Q.T@W_K@W_r
    bl_d = nc.dram_tensor("bl", [BC, D], F32, kind="ExternalInput")     # b_l rep
    bq_d = nc.dram_tensor("bq", [BC, D], F32, kind="ExternalInput")     # b_Q@W_K@W_r rep
    mask_d = nc.dram_tensor("mask", [BC, NG, GS], F32, kind="ExternalInput")
    id_d = nc.dram_tensor("ident", [BC, BC], DT, kind="ExternalInput")
    ones_d = nc.dram_tensor("ones", [1, 128], DT, kind="ExternalInput")
    out_d = nc.dram_tensor("out", [BC, D], F32, kind="ExternalOutput")

    with tile.TileContext(nc) as tc:
        with (
            tc.tile_pool(name="wp", bufs=1) as wp,
            tc.tile_pool(name="sp", bufs=1) as sp,
            tc.tile_pool(name="pT", bufs=2, space="PSUM") as pT,
            tc.tile_pool(name="pM", bufs=3, space="PSUM") as pM,
            tc.tile_pool(name="pC", bufs=3, space="PSUM") as pC,
        ):
            # --- DMAs in chain-consumption order: hist first ---
            Htg = []
            neigh_h = [sp.tile([128, 3, BC], F32, name=f"nh{ch}", tag=f"nh{ch}") for ch in range(2)]
            for g in range(NG):
                ht = sp.tile([128, DC, GS, H], DT, tag=f"ht{g}")
                nc.sync.dma_start(
                    ht[:],
                    hist_d[:, g * GS * H:(g + 1) * GS * H].rearrange(
                        "(c p) x -> p c x", p=128
                    ),
                )
                Htg.append(ht)
            for ch in range(2):
                for g in range(NG):
                    nc.vector.tensor_reduce(
                        neigh_h[ch][:, :, g * GS:(g + 1) * GS],
                        Htg[g][:, 3 * ch:3 * (ch + 1), :, 0:96], X, ALU.add,
                    )

            wlt = [wp.tile([128, DC, NH], DT, name=f"wlt{j}", tag=f"wlt{j}")
                   for j in range(2)]
            for j in range(2):
                nc.sync.dma_start(
                    wlt[j][:],
                    wlt_d[:, j * NH:(j + 1) * NH].rearrange("(c p) e -> p c e", p=128),
                )
            bl = wp.tile([BC, D], F32)
            nc.sync.dma_start(bl[:], bl_d[:])
            bq = wp.tile([BC, D], F32)
            nc.sync.dma_start(bq[:], bq_d[:])
            mask_s = wp.tile([BC, NG, GS], F32)
            nc.sync.dma_start(mask_s[:], mask_d[:])
            mask = wp.tile([BC, NG, GS, H], F32)
            nc.scalar.copy(
                mask[:], mask_s[:].unsqueeze(3).broadcast_to([BC, NG, GS, H])
            )
            ident = wp.tile([BC, BC], DT)
            nc.sync.dma_start(ident[:], id_d[:])
            onesr = wp.tile([1, 128], DT)
            nc.sync.dma_start(onesr[:], ones_d[:])
            wrt = [wp.tile([128, D], DT, name=f"wrt{k}", tag=f"wrt{k}")
                   for k in range(DC)]
            for k in range(DC):
                nc.sync.dma_start(
                    wrt[k][:],
                    wrt_d[k * 128:(k + 1) * 128, :].rearrange("p e -> p e"),
                )
            wqkr = [wp.tile([128, D], DT, name=f"wqkr{k}", tag=f"wqkr{k}")
                    for k in range(DC)]
            for k in range(DC):
                nc.sync.dma_start(
                    wqkr[k][:],
                    wqkr_d[k * 128:(k + 1) * 128, :].rearrange("p d -> p d"),
                )

            # hist0 (slot h=0 of every sample) as its own lhsT tile
            hist0 = sp.tile([128, DC, BC], DT)
            for g in range(NG):
                nc.scalar.copy(hist0[:, :, g * GS:(g + 1) * GS], Htg[g][:, :, :, 0])

            neighr = [sp.tile([128, 3, BC], DT, name=f"nr{ch}", tag=f"nr{ch}") for ch in range(2)]
            for ch in range(2):
                nc.scalar.copy(neighr[ch][:], neigh_h[ch][:])

            JS = [slice(j * NH, (j + 1) * NH) for j in range(2)]

            # --- c = neigh @ (W_l.T/96) + b_l ---
            c_sb = sp.tile([BC, D], F32)
            for j in range(2):
                ps = pM.tile([BC, NH], F32, tag="mm")
                for k in range(DC):
                    nc.tensor.matmul(
                        ps[:], neighr[k // 3][:, k % 3, :], wlt[j][:, k, :],
                        start=(k == 0), stop=(k == DC - 1),
                    )
                nc.vector.tensor_add(c_sb[:, JS[j]], ps[:], bl[:, JS[j]])

            # --- urep = hist0 @ W_r.T + c (half tiles: transposes chase) ---
            urep = [sp.tile([BC, NH], DT, name=f"urep{j}", tag=f"urep{j}")
                    for j in range(2)]
            for j in range(2):
                ps = pM.tile([BC, NH], F32, tag="mm")
                for k in range(DC):
                    nc.tensor.matmul(
                        ps[:], hist0[:, k, :], wrt[k][:, JS[j]],
                        start=(k == 0), stop=(k == DC - 1),
                    )
                nc.vector.tensor_add(urep[j][:], ps[:], c_sb[:, JS[j]])

            identf = ident[:]

            def transpose_rows(src_halves, dst_halves):
                # two [BC, 384] f16 halves -> two [128, 3, BC] f16 half tiles
                for hh in range(2):
                    pt = pT.tile([128, 3, BC], DT, tag="pt")
                    for i in range(3):
                        nc.tensor.transpose(
                            pt[:, i, :],
                            src_halves[hh][:, i * 128:(i + 1) * 128], identf,
                        )
                    nc.vector.tensor_copy(dst_halves[hh][:], pt[:])

            urepT = [sp.tile([128, 3, BC], DT, name=f"urepT{h}", tag=f"urepT{h}")
                     for h in range(2)]
            transpose_rows(urep, urepT)

            # --- u = urep @ (W_Q.T @ W_K @ W_r) + b_Q@W_K@W_r ---
            u_sb = [sp.tile([BC, NH], DT, name=f"u{j}", tag=f"u{j}")
                    for j in range(2)]
            for j in range(2):
                ps = pM.tile([BC, NH], F32, tag="mm")
                for k in range(DC):
                    nc.tensor.matmul(
                        ps[:], urepT[k // 3][:, k % 3, :], wqkr[k][:, JS[j]],
                        start=(k == 0), stop=(k == DC - 1),
                    )
                nc.vector.tensor_add(u_sb[j][:], ps[:], bq[:, JS[j]])
            uT = [sp.tile([128, 3, BC], DT, name=f"uT{h}", tag=f"uT{h}")
                  for h in range(2)]
            transpose_rows(u_sb, uT)

            # --- scores cross-product per group, diag via masked reduce ---
            scores = sp.tile([BC, H], F32)
            sc_m = sp.tile([BC, NG, GS, H], F32)
            for g in range(NG):
                ps = pC.tile([BC, GS, H], F32, tag="sc")
                for k in range(DC):
                    nc.tensor.matmul(
                        ps[:], uT[k // 3][:, k % 3, :], Htg[g][:, k, :, :],
                        start=(k == 0), stop=(k == DC - 1),
                    )
                nc.vector.scalar_tensor_tensor(
                    sc_m[:, g], ps[:], 1.0, mask[:, g],
                    op0=ALU.mult, op1=ALU.mult,
                )
            nc.vector.tensor_reduce(
                scores[:], sc_m[:].rearrange("p g i h -> p h g i"), X2, ALU.add
            )

            # --- softmax over h (1/16 scale folded into exp) ---
            mx = sp.tile([BC, 1], F32)
            nc.vector.tensor_reduce(mx[:], scores[:], X, ALU.max)
            nmx = sp.tile([BC, 1], F32)
            nc.scalar.mul(nmx[:], mx[:], -1.0 / 16.0)
            expsc = sp.tile([BC, H], DT)
            zs = sp.tile([BC, 1], F32)
            nc.scalar.activation(
                expsc[:], scores[:], ACTF.Exp,
                bias=nmx[:], scale=1.0 / 16.0, accum_out=zs[:],
            )
            recip = sp.tile([BC, 1], F32)
            nc.vector.reciprocal(recip[:], zs[:])

            # --- broadcast e over partitions; weighted sums per group ---
            e_row = sp.tile([1, BC * H], DT)
            nc.gpsimd.dma_start(e_row[:], expsc[:])
            bcast = sp.tile([128, BC, H], DT)
            for j3 in range(3):
                psb = pC.tile([128, 400], F32, tag="sc")
                nc.tensor.matmul(
                    psb[:], onesr[:], e_row[:, 400 * j3:400 * (j3 + 1)],
                    start=True, stop=True,
                )
                nc.scalar.copy(
                    bcast[:, 4 * j3:4 * (j3 + 1), :],
                    psb[:].rearrange("p (b h) -> p b h", b=GS),
                )

            # HAM warmers: results unused; each depends on a tail tensor so
            # they fire spread across the PE-idle window (exp -> e_row ->
            # bcast -> tmp slices), keeping PE activity gaps < the ~3.4us
            # HAM re-throttle window before the final out matmuls.
            ps_w = pM.tile([128, 512], F32, name="ps_w", tag="mm")
            nc.tensor.matmul(
                ps_w[0:100, 0:100], expsc[:], expsc[:], start=True, stop=True
            )
            ps_w2 = pM.tile([128, 512], F32, name="ps_w2", tag="mm")
            nc.tensor.matmul(
                ps_w2[:, 0:512], e_row[:, 0:128], e_row[:, 0:512],
                start=True, stop=True,
            )

            tmp = sp.tile([128, DC, BC, H], DT)
            for kp in range(3):
                nc.gpsimd.tensor_mul(
                    tmp[:, 2 * kp:2 * (kp + 1), 2 * GS:, :],
                    Htg[2][:, 2 * kp:2 * (kp + 1), :, :],
                    bcast[:, 2 * GS:, :]
                    .unsqueeze(1)
                    .broadcast_to([128, 2, GS, H]),
                )

            ps_wb = pM.tile([128, 512], F32, name="ps_wb", tag="mm")
            nc.tensor.matmul(
                ps_wb[0:BC, 0:400], uT[0][:, 0, :], bcast[:, 0:GS, :],
                start=True, stop=True,
            )
            for g in range(NG):
                ps_wg = pM.tile([128, 512], F32, name=f"ps_wg{g}", tag="mm")
                nc.tensor.matmul(
                    ps_wg[0:BC, 0:400],
                    uT[0][:, g, :],
                    tmp[:, g, g * GS:(g + 1) * GS, :],
                    start=True, stop=True,
                )

            # weighted sums per k-chunk-pair; out matmuls chase the reduces
            out_sb = sp.tile([BC, D], F32)
            ps_o = [pM.tile([BC, NH], F32, name=f"po{j}", tag="mm") for j in range(2)]
            hbarr = [
                sp.tile([128, 2, BC], DT, name=f"hb{kp}", tag=f"hb{kp}")
                for kp in range(3)
            ]
            hbar_f = sp.tile([128, 2, BC], F32)
            for kp in range(3):
                for g in range(2):
                    nc.vector.tensor_mul(
                        tmp[:, 2 * kp:2 * (kp + 1), g * GS:(g + 1) * GS, :],
                        Htg[g][:, 2 * kp:2 * (kp + 1), :, :],
                        bcast[:, g * GS:(g + 1) * GS, :]
                        .unsqueeze(1)
                        .broadcast_to([128, 2, GS, H]),
                    )
                nc.vector.tensor_reduce(
                    hbar_f[:], tmp[:, 2 * kp:2 * (kp + 1), :, :], X, ALU.add,
                )
                nc.scalar.copy(hbarr[kp][:], hbar_f[:])
                for j in range(2):
                    for k in range(2):
                        nc.tensor.matmul(
                            ps_o[j][:], hbarr[kp][:, k, :],
                            wrt[2 * kp + k][:, JS[j]],
                            start=(kp == 0 and k == 0),
                            stop=(kp == 2 and k == 1),
                        )
            for j in range(2):
                nc.vector.scalar_tensor_tensor(
                    out_sb[:, JS[j]], ps_o[j][:], recip[:], c_sb[:, JS[j]],
                    op0=ALU.mult, op1=ALU.add,
                )
            nc.sync.dma_start(out_d[:], out_sb[:])

    nc.compile()
    return nc


def _get_nc():
    if "nc" not in _CACHE:
        _CACHE["nc"] = _build()
    return _CACHE["nc"]


def _diag_mask():
    m = np.zeros((BC, NG, GS), np.float32)
    for s in range(BC):
        m[s, s // GS, s % GS] = 1.0
    return m


def _prep_in_maps(inputs):
    hist = np.ascontiguousarray(np.asarray(inputs["history_embedding"], dtype=np.float32))
    W_l = np.asarray(inputs["W_l"], dtype=np.float32)
    b_l = np.asarray(inputs["b_l"], dtype=np.float32)
    W_r = np.asarray(inputs["W_r"], dtype=np.float32)
    W_K = np.asarray(inputs["W_K"], dtype=np.float32)
    W_Q = np.asarray(inputs["W_Q"], dtype=np.float32)
    b_Q = np.asarray(inputs["b_Q"], dtype=np.float32)

    shared = {
        "wlt": np.ascontiguousarray(W_l.T / np.float32(96.0)).astype(np.float16),
        "wrt": np.ascontiguousarray(W_r.T).astype(np.float16),
        "wqkr": np.ascontiguousarray(W_Q.T @ (W_K @ W_r)).astype(np.float16),
        "bl": np.ascontiguousarray(np.broadcast_to(b_l, (BC, D))),
        "bq": np.ascontiguousarray(np.broadcast_to(b_Q @ (W_K @ W_r), (BC, D))),
        "ident": np.eye(BC, dtype=np.float16),
        "ones": np.ones((1, 128), np.float16),
        "mask": _diag_mask(),
    }
    in_maps = []
    for i in range(NCORES):
        m = dict(shared)
        hs = hist[i * BC:(i + 1) * BC]  # [12,100,768]
        m["hist"] = np.ascontiguousarray(
            hs.transpose(2, 0, 1).reshape(D, BC * H).astype(np.float16)
        )
        in_maps.append(m)
    return in_maps


def run_device(inputs, trace=False, **kwargs):
    """Returns (out [96,768] float32, BassKernelResults)."""
    nc = _get_nc()
    in_maps = _prep_in_maps(inputs)
    try:
        res = run_bass_kernel_spmd(
            nc, in_maps, core_ids=list(range(NCORES)), trace=trace, **kwargs
        )
    except Exception:
        # transient NRT_EXEC_UNIT_UNRECOVERABLE from a wedged device has been
        # observed on first-touch; one retry reliably recovers
        res = run_bass_kernel_spmd(
            nc, in_maps, core_ids=list(range(NCORES)), trace=trace, **kwargs
        )
    out = np.concatenate(
        [np.asarray(res.results[i]["out"], dtype=np.float32) for i in range(NCORES)],
        axis=0,
    )
    return out, res


def kernel(**inputs):
    out, _ = run_device(inputs)
    full = np.broadcast_to(out[:, None, :], (B, NCAND, D))
    return np.ascontiguousarray(full)


if __name__ == "__main__":
    rng = np.random.default_rng(0)
    ins = {
        "history_embedding": rng.standard_normal((B, H, D)).astype(np.float32),
        "candidate_news_representation": rng.standard_normal((B, NCAND, D)).astype(np.float32),
        "W_l": (rng.standard_normal((D, D)) * 0.02).astype(np.float32),
        "b_l": np.zeros(D, np.float32),
        "W_r": (rng.standard_normal((D, D)) * 0.02).astype(np.float32),
        "W_K": (rng.standard_normal((A, D)) * 0.02).astype(np.float32),
        "W_Q": (rng.standard_normal((A, D)) * 0.02).astype(np.float32),
        "b_Q": np.zeros(A, np.float32),
    }
    out = kernel(**ins)
    print("kernel ran, output", out.shape, out.dtype)

